# revision 4
# baseline (speedup 1.0000x reference)
"""Trainium2 Bass kernel for nn_MoELayer (dense MoE with top-k routing).

Strategy v2 (token parallelism, zero collectives):
  Each of the 8 cores owns a contiguous slice of 1024 tokens and computes
  the FULL MoE for that slice: gate softmax + top-2 mask, all 8 routed
  expert MLPs (dense, masked by gate weight), and both shared experts.
  The final output is purely local — no cross-core combine at all; the
  host just concatenates the 8 token slices.

  - All 10 expert MLPs (2 shared + 8 routed) run through one unified
    loop; per-token combine weights w[t, i] come from the gate phase
    (shared gates for i<2, top-2-masked routed gates for i>=2).
  - Weights are streamed from HBM in bf16 (half the traffic / SBUF) in
    H-slices of 512: per (expert, hq) step, L1 computes the h^T slice
    [512, 1024] (ReLU+bias fused on DVE), L2 accumulates the partial
    y contribution; the per-token gate weight is applied by the scalar
    engine (per-partition scale) and accumulated into an SBUF-resident
    [1024, 1024] f32 accumulator by the vector engine.
  - The gate runs in true f32 (top-k ordering must match the reference);
    DVE max8/match_replace implement the top-k mask, exactly as v1.
  - b2 biases enter via a tiny [10]x[10,O] matmul that initializes the
    accumulator (exact, and free); b1 biases ride the DVE ReLU.

  Per-core budget: tensor 10240 matmuls @512 moving = ~2.2 ms,
  weight DMA 160 MB = ~0.45 ms (overlapped), DVE/scalar ~0.5 ms each
  (overlapped). No DRAM round-trips for activations, no collectives.

Environment workarounds (this walrus/axon build): every instruction may
carry at most ONE semaphore wait (see _split_multi_waits).
"""

from contextlib import ExitStack

import numpy as np

import concourse.bass as bass
import concourse.mybir as mybir
from concourse.tile import TileContext
from concourse.masks import make_identity

# ---------------------------------------------------------------- dims
B, D, H, O = 8192, 1024, 4096, 1024
E, S = 8, 2
ES = E + S            # gate columns (shared first, then routed)
NC = 8                # cores
TOPK = 2
T = B // NC           # tokens per core
HQ = 1024             # H slice per streaming step (8-deep L2 psum chains halve the epilogue)
NHQ = H // HQ         # 8 steps
DK = D // 128         # 8 d-chunks
f32 = mybir.dt.float32
bf16 = mybir.dt.bfloat16

# ------------------------------------------------- walrus sync-wait workaround
import json as _json


def _split_multi_waits(nc):
    d = _json.loads(mybir.module_to_json_string(nc.m))
    nsplit = 0
    for fn in d["functions"]:
        for bb in fn["blocks"]:
            out = []
            for inst in bb["instructions"]:
                si = inst.get("sync_info")
                waits = (si or {}).get("on_wait") or []
                if len(waits) > 1:
                    for j, w in enumerate(waits[:-1]):
                        nop = {
                            "engine": inst["engine"],
                            "ins": [],
                            "outs": [],
                            "name": f"{inst['name']}-w{j}",
                            "opcode": "NoOp",
                            "sync_info": {"on_wait": [w], "on_update": []},
                        }
                        if "debug" in inst:
                            nop["debug"] = inst["debug"]
                        out.append(nop)
                        nsplit += 1
                    si["on_wait"] = [waits[-1]]
                out.append(inst)
            bb["instructions"] = out
    nc.m = mybir.module_from_json_string(_json.dumps(d))
    return nsplit


# ---------------------------------------------------------------- builder
def build() -> bass.Bass:
    nc = bass.Bass()
    xTf = nc.declare_dram_parameter("xTf", [D, T], f32, isOutput=False)
    wall1 = nc.declare_dram_parameter("wall1", [ES, D, H], bf16, isOutput=False)
    wall2 = nc.declare_dram_parameter("wall2", [ES, H, O], bf16, isOutput=False)
    b1c = nc.declare_dram_parameter("b1c", [128, ES * H // 128], f32, isOutput=False)
    b2all = nc.declare_dram_parameter("b2all", [ES, O], f32, isOutput=False)
    wg = nc.declare_dram_parameter("wg", [D, ES], f32, isOutput=False)
    bg = nc.declare_dram_parameter("bg", [ES, 1], f32, isOutput=False)
    y = nc.declare_dram_parameter("y", [T, O], f32, isOutput=True)

    Relu = mybir.ActivationFunctionType.Relu
    Ident = mybir.ActivationFunctionType.Identity
    Exp = mybir.ActivationFunctionType.Exp
    AX = mybir.AxisListType.X
    Add = mybir.AluOpType.add
    Max = mybir.AluOpType.max

    with TileContext(nc) as tc:
        with ExitStack() as top:
            const = top.enter_context(tc.tile_pool(name="const", bufs=1))
            xbp = top.enter_context(tc.tile_pool(name="xbp", bufs=1))
            wtp = top.enter_context(tc.tile_pool(name="wtp", bufs=1))
            accp = top.enter_context(tc.tile_pool(name="accp", bufs=1))

            # ---- constants ----
            ident = const.tile([128, 128], f32, tag="ident")
            make_identity(nc, ident)
            wg_sb = const.tile([128, DK * ES], f32, tag="wg_sb")
            for k in range(DK):
                nc.sync.dma_start(
                    out=wg_sb[:, k * ES : (k + 1) * ES],
                    in_=wg[k * 128 : (k + 1) * 128, :],
                )
            bg_sb = const.tile([ES, 1], f32, tag="bg_sb")
            nc.sync.dma_start(out=bg_sb[:], in_=bg[:])
            b1c_sb = const.tile([128, ES * H // 128], f32, tag="b1c_sb")
            nc.sync.dma_start(out=b1c_sb[:], in_=b1c[:, :])
            b2_sb = const.tile([ES, O], f32, tag="b2_sb")
            nc.sync.dma_start(out=b2_sb[:], in_=b2all[:, :])

            # per-token combine weights, token-major + transposed
            wtok = []
            for blk in range(T // 128):
                wt_b = wtp.tile([128, ES], f32, tag=f"wtok{blk}")
                wtok.append(wt_b)
            wT_sb = wtp.tile([ES, T], f32, tag="wT_sb")

            # token accumulator [T, O] f32
            acc = []
            for tT in range(T // 128):
                acc_t = accp.tile([128, O], f32, tag=f"acc{tT}")
                acc.append(acc_t)

            # ---------------- phase 0: gate + top-2 mask -----------------
            # (xf tiles also produce the bf16 x copy used by the MLPs)
            xb = []
            for k in range(DK):
                xb_k = xbp.tile([128, T], bf16, tag=f"xb{k}")
                xb.append(xb_k)
            with ExitStack() as gx:
                xfp = gx.enter_context(tc.tile_pool(name="xfp", bufs=1))
                gp = gx.enter_context(tc.tile_pool(name="gp", bufs=3))
                pg = gx.enter_context(tc.tile_pool(name="pg", bufs=2, space="PSUM"))
                pg2 = gx.enter_context(tc.tile_pool(name="pg2", bufs=2, space="PSUM"))

                xf = []
                for k in range(DK):
                    t = xfp.tile([128, T], f32, tag=f"xf{k}")
                    nc.sync.dma_start(out=t[:], in_=xTf[k * 128 : (k + 1) * 128, :])
                    nc.vector.tensor_copy(xb[k][:], t[:])  # f32 -> bf16
                    xf.append(t)

                gts = gp.tile([ES, T], f32, tag="gts")
                for tN in range(T // 512):
                    tsl = slice(tN * 512, (tN + 1) * 512)
                    psg = pg.tile([ES, 512], f32, tag="psg")
                    for k in range(DK):
                        nc.tensor.matmul(
                            psg[:],
                            lhsT=wg_sb[:, k * ES : (k + 1) * ES],
                            rhs=xf[k][:, tsl],
                            start=(k == 0),
                            stop=(k == DK - 1),
                        )
                    nc.scalar.activation(gts[:, tsl], psg[:], Ident, bias=bg_sb[:])

                for blk in range(T // 128):
                    bsl = slice(blk * 128, (blk + 1) * 128)
                    pst = pg2.tile([128, 128], f32, tag="pst")
                    nc.tensor.matmul(
                        pst[:, :ES],
                        lhsT=gts[:, bsl],
                        rhs=ident[:ES, :ES],
                        is_transpose=True,
                    )
                    gtm = gp.tile([128, ES], f32, tag="gtm")
                    nc.vector.tensor_copy(gtm[:], pst[:, :ES])
                    mx = gp.tile([128, 1], f32, tag="mx")
                    nc.vector.reduce_max(mx[:], gtm[:], axis=AX)
                    nmx = gp.tile([128, 1], f32, tag="nmx")
                    nc.vector.tensor_scalar_mul(nmx[:], mx[:], -1.0)
                    ex = gp.tile([128, ES], f32, tag="ex")
                    nc.scalar.activation(ex[:], gtm[:], Exp, bias=nmx[:])
                    sm = gp.tile([128, 1], f32, tag="sm")
                    nc.vector.reduce_sum(sm[:], ex[:], axis=AX)
                    rc = gp.tile([128, 1], f32, tag="rc")
                    nc.vector.reciprocal(rc[:], sm[:])
                    pr = gp.tile([128, ES], f32, tag="pr")
                    nc.vector.tensor_scalar_mul(pr[:], ex[:], rc[:])
                    # top-2 mask over routed columns
                    m8 = gp.tile([128, E], f32, tag="m8")
                    nc.vector.max(m8[:], pr[:, S:])
                    nc.vector.memset(m8[:, TOPK:], -1.0)
                    rep = gp.tile([128, E], f32, tag="rep")
                    nc.vector.match_replace(
                        rep[:], in_to_replace=m8[:], in_values=pr[:, S:], imm_value=0.0
                    )
                    nc.vector.tensor_copy(wtok[blk][:, :S], pr[:, :S])
                    nc.vector.tensor_sub(wtok[blk][:, S:], pr[:, S:], rep[:])
                    # transpose w [128, ES] -> [ES, 128] (for the b2 matmul)
                    pstT = pg2.tile([128, 128], f32, tag="pstT")
                    nc.tensor.matmul(
                        pstT[:ES, :],
                        lhsT=wtok[blk][:, :],
                        rhs=ident[:, :],
                        is_transpose=True,
                    )
                    nc.vector.tensor_copy(wT_sb[:, bsl], pstT[:ES, :])

            # ---------------- phase 1: expert MLPs, streamed -----------------
            with ExitStack() as mx_:
                w1p = mx_.enter_context(tc.tile_pool(name="w1p", bufs=2))
                w2p = mx_.enter_context(tc.tile_pool(name="w2p", bufs=2))
                hp = mx_.enter_context(tc.tile_pool(name="hp", bufs=2))
                tp = mx_.enter_context(tc.tile_pool(name="tp", bufs=4))
                p1 = mx_.enter_context(tc.tile_pool(name="p1", bufs=2, space="PSUM"))
                p2 = mx_.enter_context(tc.tile_pool(name="p2", bufs=4, space="PSUM"))

                # accumulator init: acc[t, o] = sum_i w[t, i] * b2[i, o].
                # Emitted after the first L1 block (not before the expert loop)
                # so the tensor queue is not head-of-line blocked waiting for
                # the gate's serial DVE chain to produce wT_sb.
                def init_acc():
                    for tT in range(T // 128):
                        for oc in range(O // 512):
                            osl = slice(oc * 512, (oc + 1) * 512)
                            pb = p1.tile([128, 512], f32, tag="pb")
                            nc.tensor.matmul(
                                pb[:],
                                lhsT=wT_sb[:, tT * 128 : (tT + 1) * 128],
                                rhs=b2_sb[:, osl],
                            )
                            nc.vector.tensor_copy(acc[tT][:, osl], pb[:])

                for e in range(ES):
                    for hq in range(NHQ):
                        hsl = slice(hq * HQ, (hq + 1) * HQ)
                        w1t = []
                        for k in range(DK):
                            t = w1p.tile([128, HQ], bf16, tag=f"w1t{k}")
                            nc.sync.dma_start(
                                out=t[:], in_=wall1[e, k * 128 : (k + 1) * 128, hsl]
                            )
                            w1t.append(t)
                        w2t = []
                        for j in range(HQ // 128):
                            t = w2p.tile([128, O], bf16, tag=f"w2t{j}")
                            nc.sync.dma_start(
                                out=t[:],
                                in_=wall2[e, hq * HQ + j * 128 : hq * HQ + (j + 1) * 128, :],
                            )
                            w2t.append(t)

                        # L1: h^T slice [HQ, T] (4 tiles of [128, T])
                        hts = []
                        for hsub in range(HQ // 128):
                            ht = hp.tile([128, T], bf16, tag=f"h{hsub}")
                            bcol = e * (H // 128) + hq * (HQ // 128) + hsub
                            for tN in range(T // 512):
                                tsl = slice(tN * 512, (tN + 1) * 512)
                                ps = p1.tile([128, 512], f32, tag="ps1")
                                for k in range(DK):
                                    nc.tensor.matmul(
                                        ps[:],
                                        lhsT=w1t[k][:, hsub * 128 : (hsub + 1) * 128],
                                        rhs=xb[k][:, tsl],
                                        start=(k == 0),
                                        stop=(k == DK - 1),
                                    )
                                # relu(x + b1) on DVE, f32 psum -> bf16 sbuf
                                nc.vector.tensor_scalar(
                                    ht[:, tsl],
                                    ps[:],
                                    b1c_sb[:, bcol : bcol + 1],
                                    0.0,
                                    Add,
                                    Max,
                                )
                            hts.append(ht)

                        if e == 0 and hq == 0:
                            init_acc()

                        # L2 partial + weighted accumulate
                        for tT in range(T // 128):
                            ttsl = slice(tT * 128, (tT + 1) * 128)
                            for oc in range(O // 512):
                                osl = slice(oc * 512, (oc + 1) * 512)
                                ps2 = p2.tile([128, 512], f32, tag="ps2")
                                for j in range(HQ // 128):
                                    nc.tensor.matmul(
                                        ps2[:],
                                        lhsT=hts[j][:, ttsl],
                                        rhs=w2t[j][:, osl],
                                        start=(j == 0),
                                        stop=(j == HQ // 128 - 1),
                                    )
                                tmp = tp.tile([128, 512], f32, tag="tmp")
                                nc.scalar.mul(tmp[:], ps2[:], wtok[tT][:, e : e + 1])
                                nc.vector.tensor_add(
                                    acc[tT][:, osl], acc[tT][:, osl], tmp[:]
                                )

            # ---------------- output ----------------
            for tT in range(T // 128):
                nc.sync.dma_start(
                    out=y[tT * 128 : (tT + 1) * 128, :], in_=acc[tT][:]
                )

    _split_multi_waits(nc)
    return nc


# ---------------------------------------------------------------- host side
_cache = {}


def _get_nc():
    if "nc" not in _cache:
        _cache["nc"] = build()
    return _cache["nc"]


def _make_in_maps(x, W1, b1, W2, b2, Ws1, bs1, Ws2, bs2, Wg, bg):
    import ml_dtypes

    bf = ml_dtypes.bfloat16
    x = np.asarray(x, np.float32)
    xT = np.ascontiguousarray(x.T)  # [D, B]
    wall1 = np.concatenate(
        [np.asarray(Ws1, np.float32), np.asarray(W1, np.float32)], axis=0
    ).astype(bf)  # [ES, D, H]
    wall2 = np.concatenate(
        [np.asarray(Ws2, np.float32), np.asarray(W2, np.float32)], axis=0
    ).astype(bf)  # [ES, H, O]
    b1all = np.concatenate(
        [np.asarray(bs1, np.float32), np.asarray(b1, np.float32)], axis=0
    )  # [ES, H]
    b1c = np.ascontiguousarray(b1all.reshape(ES * H // 128, 128).T)  # [128, ES*H/128]
    b2all = np.concatenate(
        [np.asarray(bs2, np.float32), np.asarray(b2, np.float32)], axis=0
    )  # [ES, O]
    Wg = np.asarray(Wg, np.float32)
    bgr = np.asarray(bg, np.float32).reshape(ES, 1)

    in_maps = []
    for c in range(NC):
        in_maps.append(
            {
                "xTf": np.ascontiguousarray(xT[:, c * T : (c + 1) * T]),
                "wall1": wall1,
                "wall2": wall2,
                "b1c": b1c,
                "b2all": b2all,
                "wg": Wg,
                "bg": bgr,
            }
        )
    return in_maps


_runner_cache = {}


def _get_runner():
    """Compile (once) a non-donating SPMD runner for the built Bass module."""
    if "r" in _runner_cache:
        return _runner_cache["r"]

    import jax
    from jax.experimental.shard_map import shard_map
    from jax.sharding import Mesh, NamedSharding, PartitionSpec

    from concourse import bass2jax

    nc = _get_nc()
    partition_name = nc.partition_id_tensor.name if nc.partition_id_tensor else None
    in_names, out_names, out_avals, zero_outs = [], [], [], []
    for alloc in nc.m.functions[0].allocations:
        if not isinstance(alloc, mybir.MemoryLocationSet):
            continue
        name = alloc.memorylocations[0].name
        if alloc.kind == "ExternalInput":
            if name != partition_name:
                in_names.append(name)
        elif alloc.kind == "ExternalOutput":
            shape = tuple(alloc.tensor_shape)
            dt_ = mybir.dt.np(alloc.dtype)
            out_names.append(name)
            out_avals.append(jax.core.ShapedArray(shape, dt_))
            zero_outs.append(np.zeros(shape, dt_))
    n_params = len(in_names)
    bind_names = list(in_names) + list(out_names)
    if partition_name is not None:
        bind_names.append(partition_name)

    def _body(*args):
        operands = list(args)
        if partition_name is not None:
            operands.append(bass2jax.partition_id_tensor())
        outs = bass2jax._bass_exec_p.bind(
            *operands,
            out_avals=tuple(out_avals),
            in_names=tuple(bind_names),
            out_names=tuple(out_names),
            lowering_input_output_aliases=(),
            sim_require_finite=True,
            sim_require_nnan=True,
            nc=nc,
        )
        return tuple(outs)

    devices = jax.devices()[:NC]
    mesh = Mesh(np.asarray(devices), ("core",))
    nin = n_params + len(out_names)
    fn = jax.jit(
        shard_map(
            _body,
            mesh=mesh,
            in_specs=(PartitionSpec("core"),) * nin,
            out_specs=(PartitionSpec("core"),) * len(out_names),
            check_rep=False,
        ),
        keep_unused=True,
    )
    sh = NamedSharding(mesh, PartitionSpec("core"))
    ret = (fn, in_names, out_names, zero_outs, sh)
    _runner_cache["r"] = ret
    return ret


def _stage_and_run(inputs):
    """Returns (device output arrays tuple, fn, staged args, out_names)."""
    import jax

    in_maps = _make_in_maps(**{k: v for k, v in inputs.items() if k != "k"})
    fn, in_names, out_names, zero_outs, sh = _get_runner()
    concat_in = [
        np.concatenate([np.asarray(in_maps[c][n]) for c in range(NC)], axis=0)
        for n in in_names
    ]
    concat_zeros = [
        np.zeros((NC * z.shape[0], *z.shape[1:]), z.dtype) for z in zero_outs
    ]
    args = [jax.device_put(a, sh) for a in concat_in + concat_zeros]
    jax.block_until_ready(args)
    out_arrs = fn(*args)
    jax.block_until_ready(out_arrs)
    return out_arrs, fn, args, out_names


def kernel(x, W1, b1, W2, b2, Ws1, bs1, Ws2, bs2, Wg, bg, k):
    assert int(k) == TOPK
    inputs = dict(x=x, W1=W1, b1=b1, W2=W2, b2=b2, Ws1=Ws1, bs1=bs1,
                  Ws2=Ws2, bs2=bs2, Wg=Wg, bg=bg, k=k)
    out_arrs, _fn, _args, out_names = _stage_and_run(inputs)
    return np.asarray(out_arrs[out_names.index("y")])


def bench(inputs, iters=8):
    """Run once for output, then time repeat executions with device-resident
    inputs. Returns (output, min wall ns per run)."""
    import time

    import jax

    out_arrs, fn, args, out_names = _stage_and_run(inputs)
    times = []
    for _ in range(iters):
        t0 = time.perf_counter()
        jax.block_until_ready(fn(*args))
        times.append(time.perf_counter() - t0)
    times.sort()
    print(f"bench times (s): min={times[0]:.4f} med={times[len(times)//2]:.4f} max={times[-1]:.4f}", flush=True)
    result = np.asarray(out_arrs[out_names.index("y")])
    return result, times[0] * 1e9
